# revision 11
# baseline (speedup 1.0000x reference)
"""Trainium2 Bass kernel for nn_NodeBlock (GNN message passing), 8-core SPMD.

Sharding: edges sorted by destination node (col); destination-node space is
split into 8 contiguous 128-aligned, edge-balanced ranges (one per core), so
each core computes complete segment sums for the nodes it owns — no
all-reduce of node aggregates is needed. BatchNorm batch statistics (train
mode) are global over the edge/node batch, so each pass runs as: stats pass →
tiny all-reduce (8 cores) → fold BN scale/shift into layer-2 weights →
recompute+forward pass. The scatter (segment mean) is a matmul with on-device
built one-hot matrices into per-128-node-window PSUM accumulators; 1/deg is
folded into the last edge-MLP layer's output scale and b3 is applied
post-aggregation with a degree>0 mask (rank-1 matmul), matching
scatter-mean's max(cnt,1) semantics.
"""

import numpy as np
from contextlib import ExitStack

import ml_dtypes

N_NODES = 50000
N_EDGES = 1600000
PD = 3
AD = 4
ED = 64
HD = 128
BN_EPS = 1e-5

WIN = 128          # nodes per scatter window
W_PER_CORE = 52    # windows per core (uniform across cores)
NODES_C = W_PER_CORE * WIN   # 6656 node slots per core
NT_TILE = 512      # nodes per node-MLP tile
SENTINEL = 255.0   # one-hot offset for padding edges (matches nothing in 0..127)

BF16 = ml_dtypes.bfloat16


# ---------------------------------------------------------------------------
# host preprocessing
# ---------------------------------------------------------------------------

def _prep(pos, ang, edge_index, edge_attr, params):
    pos = np.asarray(pos, np.float32)
    ang = np.asarray(ang, np.float32)
    ei = np.asarray(edge_index)
    ea = np.asarray(edge_attr, np.float32)
    row = ei[0].astype(np.int64)
    col = ei[1].astype(np.int64)

    deg = np.bincount(col, minlength=N_NODES)
    order = np.argsort(col, kind="stable")
    col_s = col[order]
    row_s = row[order]

    n_win = (N_NODES + WIN - 1) // WIN  # 391
    wstart = np.searchsorted(col_s, np.arange(n_win + 1) * WIN)
    wcnt = np.diff(wstart)
    wchunks = np.maximum((wcnt + 127) // 128, 1)

    # core cuts: contiguous window ranges, edge-balanced, each <= W_PER_CORE
    cuts = [0]
    target = N_EDGES / 8
    cum = np.cumsum(wcnt)
    for i in range(1, 8):
        ideal = int(np.searchsorted(cum, target * i))
        lo = max(cuts[-1] + 1, n_win - (8 - i) * W_PER_CORE)
        hi = min(cuts[-1] + W_PER_CORE, n_win - (8 - i))
        cuts.append(int(np.clip(ideal, lo, hi)))
    cuts.append(n_win)
    assert all(0 < cuts[i + 1] - cuts[i] <= W_PER_CORE for i in range(8)), cuts

    c_w = int(max(wchunks.max(), 4))
    c_w = (c_w + 3) // 4 * 4          # window = whole number of 512-edge tiles
    e_c = W_PER_CORE * c_w * 128      # padded edge slots per core

    src = np.full((8, e_c), -1, np.int64)
    loff = np.full((8, e_c), SENTINEL, np.float32)
    for c in range(8):
        for lw, gw in enumerate(range(cuts[c], cuts[c + 1])):
            a, b = int(wstart[gw]), int(wstart[gw + 1])
            n = b - a
            base = lw * c_w * 128
            assert n <= c_w * 128
            src[c, base:base + n] = np.arange(a, b)
            loff[c, base:base + n] = (col_s[a:b] - gw * WIN).astype(np.float32)

    smask = src >= 0
    sidx = np.where(smask, src, 0)
    flat = sidx.reshape(-1)

    def gather_T(table_T):
        d = table_T.shape[0]
        g = table_T[:, flat].reshape(d, 8, e_c).transpose(1, 0, 2)
        return np.where(smask[:, None, :], g, 0.0).astype(np.float32)

    ea_T = gather_T(np.ascontiguousarray(ea[order].T))      # [8, 64, e_c]
    pr_T = np.zeros((8, 4, e_c), np.float32)                # rows 0:3 pos, row3=0
    pr_T[:, 0:3, :] = gather_T(np.ascontiguousarray(pos[row_s].T))
    ar_T = gather_T(np.ascontiguousarray(ang[row_s].T))     # [8, 4, e_c]

    invd_full = np.zeros(N_NODES, np.float32)
    invd_full[deg > 0] = 1.0 / deg[deg > 0]
    invd = np.where(smask, invd_full[col_s[flat]].reshape(8, e_c), 0.0)

    node_pos_T = np.zeros((8, 4, NODES_C), np.float32)   # row0=0 pad
    node_ang_T = np.zeros((8, 4, NODES_C), np.float32)
    node_pos32 = np.zeros((8, 3, NODES_C), np.float32)
    node_ang32 = np.zeros((8, 4, NODES_C), np.float32)
    mask_row = np.zeros((8, 1, NODES_C), np.float32)
    node_spans = []
    for c in range(8):
        g0, g1 = cuts[c] * WIN, min(cuts[c + 1] * WIN, N_NODES)
        n = g1 - g0
        node_spans.append((g0, g1))
        node_pos_T[c, 1:4, :n] = pos[g0:g1].T
        node_ang_T[c, :, :n] = ang[g0:g1].T
        node_pos32[c, :, :n] = pos[g0:g1].T
        node_ang32[c, :, :n] = ang[g0:g1].T
        mask_row[c, 0, :n] = (deg[g0:g1] > 0).astype(np.float32)

    def g(p, k):
        return np.asarray(p[k], np.float32)

    def empack(p, in0_rows, din0):
        w1 = g(p, "W1")
        w1t = np.zeros((68, HD), np.float32)
        w1t[0:ED, :] = w1[:, din0:].T
        w1t[ED:ED + din0, :] = w1[:, :din0].T
        return dict(
            w1t=w1t, b1=g(p, "b1").reshape(HD, 1),
            gamma=g(p, "gamma").reshape(HD, 1), beta=g(p, "beta").reshape(HD, 1),
            w2t=np.ascontiguousarray(g(p, "W2").T), b2=g(p, "b2").reshape(HD, 1),
            w3t=np.ascontiguousarray(g(p, "W3").T), b3r=g(p, "b3").reshape(1, ED),
        )

    def npack(p, din0, pad0):
        w1 = g(p, "W1")
        w1na = np.zeros((4, HD), np.float32)
        w1na[pad0:pad0 + din0, :] = w1[:, :din0].T
        return dict(
            w1na=w1na, w1nb=np.ascontiguousarray(w1[:, din0:].T),
            b1=g(p, "b1").reshape(HD, 1),
            gamma=g(p, "gamma").reshape(HD, 1), beta=g(p, "beta").reshape(HD, 1),
            w2t=np.ascontiguousarray(g(p, "W2").T), b2=g(p, "b2").reshape(HD, 1),
            w3t=np.ascontiguousarray(g(p, "W3").T), b3=g(p, "b3").reshape(-1, 1),
        )

    packs = {"ep": empack(params["pos1"], 0, PD), "ea": empack(params["ang1"], 0, AD),
             "np": npack(params["pos2"], PD, 1), "na": npack(params["ang2"], AD, 0)}

    meta = dict(c_w=c_w, e_c=e_c, cuts=cuts, node_spans=node_spans)

    fp32_keys = {"b1", "b2", "b3", "gamma", "beta"}
    per_core = []
    for c in range(8):
        m = {
            "ea_t": ea_T[c].astype(BF16),
            "pr_t": pr_T[c].astype(BF16),
            "ar_t": ar_T[c].astype(BF16),
            "loff": np.ascontiguousarray(loff[c].reshape(-1, 128).T).astype(BF16),
            "invd": np.ascontiguousarray(invd[c].reshape(-1, 128).T).astype(BF16),
            "mask_row": mask_row[c].astype(BF16),
            "node_pos_t": node_pos_T[c].astype(BF16),
            "node_ang_t": node_ang_T[c].astype(BF16),
            "node_pos32": node_pos32[c],
            "node_ang32": node_ang32[c],
        }
        for tag, w in packs.items():
            for k, v in w.items():
                m[f"{tag}_{k}"] = np.asarray(v, np.float32 if k in fp32_keys else BF16)
        per_core.append(m)
    return per_core, meta


# ---------------------------------------------------------------------------
# walrus workaround: split instructions carrying >1 semaphore wait
# ---------------------------------------------------------------------------

def _split_multi_waits(nc, mybir, max_waits=1):
    n = 0
    for fn in nc.m.functions:
        for bb in fn.blocks:
            insts = bb.instructions
            i = 0
            while i < len(insts):
                ins = insts[i]
                si = ins.sync_info
                if si is not None and si.on_wait and len(si.on_wait) > max_waits:
                    waits = list(si.on_wait)
                    nops = [
                        mybir.InstNoOp(
                            name=f"waitnop-{nc.next_id()}",
                            engine=ins.engine, bass_nofuse=True, ins=[], outs=[],
                            sync_info=mybir.SyncInfo(on_wait=[w], on_update=[]),
                        )
                        for w in waits[:-max_waits]
                    ]
                    ins.sync_info = mybir.SyncInfo(
                        on_wait=waits[-max_waits:], on_update=list(si.on_update or []))
                    insts[i:i] = nops
                    i += len(nops)
                    n += 1
                i += 1
    return n


# ---------------------------------------------------------------------------
# device program
# ---------------------------------------------------------------------------

def _build(meta):
    import concourse.bass as bass
    import concourse.mybir as mybir
    from concourse.tile import TileContext

    fp32 = mybir.dt.float32
    bf16 = mybir.dt.bfloat16
    AF = mybir.ActivationFunctionType
    ALU = mybir.AluOpType

    c_w = meta["c_w"]
    e_c = meta["e_c"]
    n_etile = e_c // 512
    tiles_per_win = c_w // 4
    n_ntile = NODES_C // NT_TILE

    nc = bass.Bass("TRN2", num_devices=8)

    def din(name, shape, dt=bf16):
        return nc.dram_tensor(name, list(shape), dt, kind="ExternalInput")

    ea_t = din("ea_t", (ED, e_c))
    pr_t = din("pr_t", (4, e_c))
    ar_t = din("ar_t", (4, e_c))
    loff_d = din("loff", (128, e_c // 128))
    invd_d = din("invd", (128, e_c // 128))
    mask_d = din("mask_row", (1, NODES_C))
    npos_d = din("node_pos_t", (4, NODES_C))
    nang_d = din("node_ang_t", (4, NODES_C))
    npos32_d = din("node_pos32", (3, NODES_C), fp32)
    nang32_d = din("node_ang32", (4, NODES_C), fp32)

    W = {}
    for tag in ("ep", "ea"):
        W[f"{tag}_w1t"] = din(f"{tag}_w1t", (68, HD))
        W[f"{tag}_w2t"] = din(f"{tag}_w2t", (HD, HD))
        W[f"{tag}_w3t"] = din(f"{tag}_w3t", (HD, ED))
        W[f"{tag}_b3r"] = din(f"{tag}_b3r", (1, ED))
    for tag in ("np", "na"):
        dout = PD if tag == "np" else AD
        W[f"{tag}_w1na"] = din(f"{tag}_w1na", (4, HD))
        W[f"{tag}_w1nb"] = din(f"{tag}_w1nb", (ED, HD))
        W[f"{tag}_w2t"] = din(f"{tag}_w2t", (HD, HD))
        W[f"{tag}_w3t"] = din(f"{tag}_w3t", (HD, dout))
        W[f"{tag}_b3"] = din(f"{tag}_b3", (dout, 1), fp32)
    for tag in ("ep", "ea", "np", "na"):
        for k in ("b1", "b2", "gamma", "beta"):
            W[f"{tag}_{k}"] = din(f"{tag}_{k}", (HD, 1), fp32)

    out_pos = nc.dram_tensor("out_pos", [PD, NODES_C], fp32, kind="ExternalOutput")
    out_ang = nc.dram_tensor("out_ang", [AD, NODES_C], fp32, kind="ExternalOutput")

    cc_e_in = nc.dram_tensor("cc_e_in", [HD, 4], fp32)
    cc_e_out = nc.dram_tensor("cc_e_out", [HD, 4], fp32)
    cc_n_in = nc.dram_tensor("cc_n_in", [HD, 4], fp32)
    cc_n_out = nc.dram_tensor("cc_n_out", [HD, 4], fp32)

    with TileContext(nc) as tc:
        with ExitStack() as ctx:
            pers = ctx.enter_context(tc.tile_pool(name="pers", bufs=1))
            work = ctx.enter_context(tc.tile_pool(name="work", bufs=3))
            psum = ctx.enter_context(tc.tile_pool(name="psum", bufs=2, space="PSUM"))
            psum1 = ctx.enter_context(tc.tile_pool(name="psum1", bufs=1, space="PSUM"))

            # ---- persistent SBUF state ----
            loff_sb = pers.tile([128, e_c // 128], bf16, tag="loff")
            invd_sb = pers.tile([128, e_c // 128], bf16, tag="invd")
            mask_sb = pers.tile([1, NODES_C], bf16, tag="mask")
            npos_sb = pers.tile([4, NODES_C], bf16, tag="npos")
            nang_sb = pers.tile([4, NODES_C], bf16, tag="nang")
            for sb, d in ((loff_sb, loff_d), (invd_sb, invd_d), (mask_sb, mask_d),
                          (npos_sb, npos_d), (nang_sb, nang_d)):
                nc.sync.dma_start(sb[:], d[:])

            wsb = {}
            for k, d in W.items():
                t = pers.tile(list(d.shape), d.dtype, tag=f"w_{k}", name=f"w_{k}")
                nc.sync.dma_start(t[:], d[:])
                wsb[k] = t

            # iota row (value j at [p, j]) and identity matrix, both bf16
            iota_i = pers.tile([128, 128], mybir.dt.int32, tag="iota_i")
            nc.gpsimd.iota(iota_i[:], pattern=[[1, 128]], base=0, channel_multiplier=0)
            iota_bf = pers.tile([128, 128], bf16, tag="iota_bf")
            nc.vector.tensor_copy(iota_bf[:], iota_i[:])
            iota_p = pers.tile([128, 128], mybir.dt.int32, tag="iota_p")
            nc.gpsimd.iota(iota_p[:], pattern=[[0, 128]], base=0, channel_multiplier=1)
            iota_pbf = pers.tile([128, 128], bf16, tag="iota_pbf")
            nc.vector.tensor_copy(iota_pbf[:], iota_p[:])
            eps_ap = pers.tile([HD, 1], fp32, tag="eps_ap")
            nc.gpsimd.memset(eps_ap[:], BN_EPS)
            ident = pers.tile([128, 128], bf16, tag="ident")
            nc.vector.scalar_tensor_tensor(
                out=ident[:], in0=iota_bf[:], scalar=0.0, in1=iota_pbf[:],
                op0=ALU.bypass, op1=ALU.is_equal)

            # stats slots
            sumP = pers.tile([HD, n_etile], fp32, tag="sumP")
            sqP = pers.tile([HD, n_etile], fp32, tag="sqP")
            sumA = pers.tile([HD, n_etile], fp32, tag="sumA")
            sqA = pers.tile([HD, n_etile], fp32, tag="sqA")
            stats_e = pers.tile([HD, 4], fp32, tag="stats_e")
            gstats_e = pers.tile([HD, 4], fp32, tag="gstats_e")
            nsum = pers.tile([HD, 2 * n_ntile], fp32, tag="nsum")
            nsq = pers.tile([HD, 2 * n_ntile], fp32, tag="nsq")
            stats_n = pers.tile([HD, 4], fp32, tag="stats_n")
            gstats_n = pers.tile([HD, 4], fp32, tag="gstats_n")

            w2_eff = {t: pers.tile([HD, HD], bf16, tag=f"w2eff_{t}",
                                   name=f"w2eff_{t}")
                      for t in ("ep", "ea", "np", "na")}
            b2_eff = {t: pers.tile([HD, 1], fp32, tag=f"b2eff_{t}",
                                   name=f"b2eff_{t}")
                      for t in ("ep", "ea", "np", "na")}

            h1n = {"np": pers.tile([HD, NODES_C], bf16, tag="h1np", name="h1np"),
                   "na": pers.tile([HD, NODES_C], bf16, tag="h1na", name="h1na")}
            aggT = {"ep": pers.tile([ED, NODES_C], bf16, tag="aggP", name="aggP"),
                    "ea": pers.tile([ED, NODES_C], bf16, tag="aggA", name="aggA")}

            # ---------------- edge pass A: BN stats ----------------
            def load_edge_tile(t):
                tp = work.tile([68, 512], bf16, tag="tileP")
                ta = work.tile([68, 512], bf16, tag="tileA")
                s = t * 512
                nc.sync.dma_start(tp[0:64, :], ea_t[:, s:s + 512])
                nc.sync.dma_start(tp[64:68, :], pr_t[:, s:s + 512])
                nc.sync.dma_start(ta[64:68, :], ar_t[:, s:s + 512])
                nc.vector.tensor_copy(ta[0:64, :], tp[0:64, :])
                return tp, ta

            for t in range(n_etile):
                tp, ta = load_edge_tile(t)
                for tag, rhs, ss, qq in (("ep", tp, sumP, sqP), ("ea", ta, sumA, sqA)):
                    z = psum.tile([HD, 512], fp32, tag="z")
                    nc.tensor.matmul(z[:], wsb[f"{tag}_w1t"][:], rhs[:],
                                     start=True, stop=True)
                    h = work.tile([HD, 512], bf16, tag=f"hA_{tag}")
                    nc.scalar.activation(h[:], z[:], AF.Relu,
                                         bias=wsb[f"{tag}_b1"][:, 0:1],
                                         accum_out=ss[:, t:t + 1])
                    trash = work.tile([HD, 512], bf16, tag="trashA")
                    nc.vector.scalar_tensor_tensor(
                        out=trash[:], in0=h[:], scalar=0.0, in1=h[:],
                        op0=ALU.bypass, op1=ALU.mult, accum_out=qq[:, t:t + 1])

            for src_t, cidx in ((sumP, 0), (sqP, 1), (sumA, 2), (sqA, 3)):
                nc.vector.tensor_reduce(stats_e[:, cidx:cidx + 1], src_t[:],
                                        axis=mybir.AxisListType.X, op=ALU.add)
            nc.sync.dma_start(cc_e_in[:], stats_e[:])
            nc.gpsimd.collective_compute(
                "AllReduce", ALU.add, replica_groups=[list(range(8))],
                ins=[cc_e_in[:]], outs=[cc_e_out[:]])
            nc.sync.dma_start(gstats_e[:], cc_e_out[:])

            # ---------------- fold BN into W2/b2 ----------------
            def fold(tag, gstats, c0, inv_n):
                mu = pers.tile([HD, 1], fp32, tag=f"mu_{tag}")
                nc.scalar.mul(mu[:], gstats[:, c0:c0 + 1], inv_n)
                msq = pers.tile([HD, 1], fp32, tag=f"msq_{tag}")
                nc.scalar.mul(msq[:], gstats[:, c0 + 1:c0 + 2], inv_n)
                mu2 = pers.tile([HD, 1], fp32, tag=f"mu2_{tag}")
                nc.vector.scalar_tensor_tensor(
                    out=mu2[:], in0=mu[:], scalar=0.0, in1=mu[:],
                    op0=ALU.bypass, op1=ALU.mult)
                var = pers.tile([HD, 1], fp32, tag=f"var_{tag}")
                nc.vector.scalar_tensor_tensor(
                    out=var[:], in0=msq[:], scalar=0.0, in1=mu2[:],
                    op0=ALU.bypass, op1=ALU.subtract)
                sig = pers.tile([HD, 1], fp32, tag=f"sig_{tag}")
                nc.scalar.activation(sig[:], var[:], AF.Sqrt, bias=eps_ap[:, 0:1])
                rs = pers.tile([HD, 1], fp32, tag=f"rs_{tag}")
                nc.vector.reciprocal(rs[:], sig[:])
                gsi = pers.tile([HD, 1], fp32, tag=f"gsi_{tag}")
                nc.vector.scalar_tensor_tensor(
                    out=gsi[:], in0=rs[:], scalar=0.0, in1=wsb[f"{tag}_gamma"][:],
                    op0=ALU.bypass, op1=ALU.mult)
                nc.scalar.activation(w2_eff[tag][:], wsb[f"{tag}_w2t"][:], AF.Copy,
                                     scale=gsi[:, 0:1])
                mgsi = pers.tile([HD, 1], fp32, tag=f"mgsi_{tag}")
                nc.vector.scalar_tensor_tensor(
                    out=mgsi[:], in0=mu[:], scalar=0.0, in1=gsi[:],
                    op0=ALU.bypass, op1=ALU.mult)
                v = pers.tile([HD, 1], fp32, tag=f"v_{tag}")
                nc.vector.scalar_tensor_tensor(
                    out=v[:], in0=wsb[f"{tag}_beta"][:], scalar=0.0, in1=mgsi[:],
                    op0=ALU.bypass, op1=ALU.subtract)
                vb = pers.tile([HD, 1], bf16, tag=f"vb_{tag}")
                nc.vector.tensor_copy(vb[:], v[:])
                pb = psum1.tile([HD, 1], fp32, tag="scratch1")
                nc.tensor.matmul(pb[:], wsb[f"{tag}_w2t"][:], vb[:],
                                 start=True, stop=True)
                nc.vector.scalar_tensor_tensor(
                    out=b2_eff[tag][:], in0=pb[:], scalar=0.0,
                    in1=wsb[f"{tag}_b2"][:], op0=ALU.bypass, op1=ALU.add)

            fold("ep", gstats_e, 0, 1.0 / N_EDGES)
            fold("ea", gstats_e, 2, 1.0 / N_EDGES)

            # ---------------- edge pass B + scatter ----------------
            for w in range(W_PER_CORE):
                aggps = {"ep": psum1.tile([128, ED], fp32, tag="aggps_p",
                                          name="aggps_p"),
                         "ea": psum1.tile([128, ED], fp32, tag="aggps_a",
                                          name="aggps_a")}
                for tt in range(tiles_per_win):
                    t = w * tiles_per_win + tt
                    tp, ta = load_edge_tile(t)
                    u_sb = {}
                    for tag, rhs in (("ep", tp), ("ea", ta)):
                        z1 = psum.tile([HD, 512], fp32, tag="z")
                        nc.tensor.matmul(z1[:], wsb[f"{tag}_w1t"][:], rhs[:],
                                         start=True, stop=True)
                        h1 = work.tile([HD, 512], bf16, tag=f"h1B_{tag}")
                        nc.scalar.activation(h1[:], z1[:], AF.Relu,
                                             bias=wsb[f"{tag}_b1"][:, 0:1])
                        z2 = psum.tile([HD, 512], fp32, tag="z")
                        nc.tensor.matmul(z2[:], w2_eff[tag][:], h1[:],
                                         start=True, stop=True)
                        h2 = work.tile([HD, 512], bf16, tag=f"h2B_{tag}")
                        nc.scalar.activation(h2[:], z2[:], AF.Relu,
                                             bias=b2_eff[tag][:, 0:1])
                        ups = psum.tile([128, 4, ED], fp32, tag="ups")
                        for cch in range(4):
                            nc.tensor.matmul(ups[:, cch, :],
                                             h2[:, 128 * cch:128 * (cch + 1)],
                                             wsb[f"{tag}_w3t"][:],
                                             start=True, stop=True)
                        usb = work.tile([128, 4, ED], bf16, tag=f"usbB_{tag}")
                        iv = invd_sb[:, 4 * t:4 * t + 4].unsqueeze(2) \
                            .broadcast_to([128, 4, ED])
                        nc.vector.scalar_tensor_tensor(
                            out=usb[:], in0=iv, scalar=0.0, in1=ups[:],
                            op0=ALU.bypass, op1=ALU.mult)
                        u_sb[tag] = usb
                    for cch in range(4):
                        ct = 4 * t + cch
                        oh = work.tile([128, 128], bf16, tag="oh")
                        lo = loff_sb[:, ct:ct + 1].broadcast_to([128, 128])
                        nc.vector.scalar_tensor_tensor(
                            out=oh[:], in0=lo, scalar=0.0, in1=iota_bf[:],
                            op0=ALU.bypass, op1=ALU.is_equal)
                        first = (tt == 0 and cch == 0)
                        for tag in ("ep", "ea"):
                            nc.tensor.matmul(aggps[tag][:], oh[:],
                                             u_sb[tag][:, cch, :],
                                             start=first, stop=False,
                                             skip_group_check=True)
                for tag in ("ep", "ea"):
                    nc.tensor.matmul(aggps[tag][:], mask_sb[:, w * WIN:(w + 1) * WIN],
                                     wsb[f"{tag}_b3r"][:], start=False, stop=True,
                                     skip_group_check=True)
                    asb = work.tile([128, ED], bf16, tag=f"asb_{tag}")
                    nc.scalar.copy(asb[:], aggps[tag][:])
                    tps = psum1.tile([ED, 128], bf16, tag="scratch1", name="tps")
                    nc.tensor.transpose(tps[:], asb[:], ident[:])
                    nc.scalar.copy(aggT[tag][:, w * WIN:(w + 1) * WIN], tps[:])

            # ---------------- node pass A ----------------
            for i in range(n_ntile):
                s = i * NT_TILE
                for tag, nf, agg in (("np", npos_sb, aggT["ep"]),
                                     ("na", nang_sb, aggT["ea"])):
                    z = psum.tile([HD, NT_TILE], fp32, tag="z")
                    nc.tensor.matmul(z[:], wsb[f"{tag}_w1na"][:],
                                     nf[:, s:s + NT_TILE], start=True, stop=False)
                    nc.tensor.matmul(z[:], wsb[f"{tag}_w1nb"][:],
                                     agg[:, s:s + NT_TILE], start=False, stop=True)
                    col = (0 if tag == "np" else n_ntile) + i
                    nc.scalar.activation(h1n[tag][:, s:s + NT_TILE], z[:], AF.Relu,
                                         bias=wsb[f"{tag}_b1"][:, 0:1],
                                         accum_out=nsum[:, col:col + 1])
                    trash = work.tile([HD, NT_TILE], bf16, tag="trashA")
                    nc.vector.scalar_tensor_tensor(
                        out=trash[:], in0=h1n[tag][:, s:s + NT_TILE], scalar=0.0,
                        in1=h1n[tag][:, s:s + NT_TILE],
                        op0=ALU.bypass, op1=ALU.mult, accum_out=nsq[:, col:col + 1])

            for cidx, (src_t, a) in enumerate(((nsum, 0), (nsq, 0),
                                               (nsum, n_ntile), (nsq, n_ntile))):
                nc.vector.tensor_reduce(stats_n[:, cidx:cidx + 1],
                                        src_t[:, a:a + n_ntile],
                                        axis=mybir.AxisListType.X, op=ALU.add)
            nc.sync.dma_start(cc_n_in[:], stats_n[:])
            nc.gpsimd.collective_compute(
                "AllReduce", ALU.add, replica_groups=[list(range(8))],
                ins=[cc_n_in[:]], outs=[cc_n_out[:]])
            nc.sync.dma_start(gstats_n[:], cc_n_out[:])

            fold("np", gstats_n, 0, 1.0 / N_NODES)
            fold("na", gstats_n, 2, 1.0 / N_NODES)

            # ---------------- node pass B ----------------
            for i in range(n_ntile):
                s = i * NT_TILE
                for tag, dout, res_d, outd in (("np", PD, npos32_d, out_pos),
                                               ("na", AD, nang32_d, out_ang)):
                    z2 = psum.tile([HD, NT_TILE], fp32, tag="z")
                    nc.tensor.matmul(z2[:], w2_eff[tag][:],
                                     h1n[tag][:, s:s + NT_TILE],
                                     start=True, stop=True)
                    h2 = work.tile([HD, NT_TILE], bf16, tag=f"h2B_{tag}")
                    nc.scalar.activation(h2[:], z2[:], AF.Relu,
                                         bias=b2_eff[tag][:, 0:1])
                    u3 = psum.tile([dout, NT_TILE], fp32, tag="ups")
                    nc.tensor.matmul(u3[:], wsb[f"{tag}_w3t"][:], h2[:],
                                     start=True, stop=True)
                    res = work.tile([dout, NT_TILE], fp32, tag=f"res_{tag}")
                    nc.sync.dma_start(res[:], res_d[:, s:s + NT_TILE])
                    ob = work.tile([dout, NT_TILE], fp32, tag=f"ob_{tag}")
                    nc.vector.scalar_tensor_tensor(
                        out=ob[:], in0=u3[:], scalar=wsb[f"{tag}_b3"][:, 0:1],
                        in1=res[:], op0=ALU.add, op1=ALU.add)
                    nc.sync.dma_start(outd[:, s:s + NT_TILE], ob[:])

    n_split = _split_multi_waits(nc, mybir)
    return nc, n_split


# ---------------------------------------------------------------------------
# entry point
# ---------------------------------------------------------------------------

def _exec_spmd(nc, in_maps, iters=1):
    """Mirror bass2jax.run_bass_via_pjrt's 8-core path, with inputs held
    device-resident so repeated executions time the kernel itself."""
    import time
    import jax
    import concourse.mybir as mybir
    from concourse import bass2jax
    from jax.sharding import Mesh, PartitionSpec, NamedSharding
    from jax.experimental.shard_map import shard_map

    bass2jax.install_neuronx_cc_hook()
    in_names, out_names, out_avals, zero_outs = [], [], [], []
    partition_name = nc.partition_id_tensor.name if nc.partition_id_tensor else None
    for alloc in nc.m.functions[0].allocations:
        if not isinstance(alloc, mybir.MemoryLocationSet):
            continue
        name = alloc.memorylocations[0].name
        if alloc.kind == "ExternalInput":
            if name != partition_name:
                in_names.append(name)
        elif alloc.kind == "ExternalOutput":
            shape = tuple(alloc.tensor_shape)
            dtype = mybir.dt.np(alloc.dtype)
            out_names.append(name)
            out_avals.append(jax.core.ShapedArray(shape, dtype))
            zero_outs.append(np.zeros(shape, dtype))
    n_params = len(in_names)
    n_outs = len(out_avals)
    in_names_all = list(in_names) + list(out_names)
    if partition_name is not None:
        in_names_all.append(partition_name)
    donate = tuple(range(n_params, n_params + n_outs))

    def _body(*args):
        operands = list(args)
        if partition_name is not None:
            operands.append(bass2jax.partition_id_tensor())
        outs = bass2jax._bass_exec_p.bind(
            *operands, out_avals=tuple(out_avals), in_names=tuple(in_names_all),
            out_names=tuple(out_names), lowering_input_output_aliases=(),
            sim_require_finite=True, sim_require_nnan=True, nc=nc)
        return tuple(outs)

    devices = jax.devices()[:8]
    mesh = Mesh(np.asarray(devices), ("core",))
    sharded = jax.jit(
        shard_map(_body, mesh=mesh, in_specs=(PartitionSpec("core"),) * (n_params + n_outs),
                  out_specs=(PartitionSpec("core"),) * n_outs, check_rep=False),
        donate_argnums=donate, keep_unused=True)
    sh = NamedSharding(mesh, PartitionSpec("core"))
    concat_in = [
        jax.device_put(
            np.concatenate([np.asarray(in_maps[c][k]) for c in range(8)], axis=0), sh)
        for k in in_names]
    jax.block_until_ready(concat_in)

    times = []
    out_arrs = None
    for it in range(iters):
        zs = [np.zeros((8 * z.shape[0], *z.shape[1:]), z.dtype) for z in zero_outs]
        t0 = time.perf_counter()
        out_arrs = sharded(*concat_in, *zs)
        jax.block_until_ready(out_arrs)
        times.append(time.perf_counter() - t0)
    results = [
        {name: np.asarray(out_arrs[i]).reshape(8, *out_avals[i].shape)[c]
         for i, name in enumerate(out_names)}
        for c in range(8)]
    return results, times


def kernel(pos, ang, edge_index, edge_attr, params):
    import os

    per_core, meta = _prep(pos, ang, edge_index, edge_attr, params)
    nc, _ = _build(meta)
    iters = int(os.environ.get("BASS_BENCH_ITERS", "1"))
    results, times = _exec_spmd(nc, per_core, iters=iters)
    if len(times) > 1:
        best = min(times[1:])
        print(f"exec wall times (s): {['%.4f' % t for t in times]}")
        print(f"HW exec time: {best*1e9:.0f} ns")

    class _R:
        pass
    res = _R()
    res.results = results

    u = np.zeros((N_NODES, PD), np.float32)
    phi = np.zeros((N_NODES, AD), np.float32)
    for c in range(8):
        g0, g1 = meta["node_spans"][c]
        n = g1 - g0
        u[g0:g1] = np.asarray(res.results[c]["out_pos"], np.float32)[:, :n].T
        phi[g0:g1] = np.asarray(res.results[c]["out_ang"], np.float32)[:, :n].T
    return u, phi
